# revision 3
# baseline (speedup 1.0000x reference)
"""Adaptive average pooling (8,384,384,64) NHWC -> (8,7,7,64) on 8 TRN2 NeuronCores.

Pure data parallel: one batch sample per core, no collectives. Per core:
  - W is covered by 5 nearly non-overlapping spans [0,110) [110,220)
    [220,275) [275,330) [329,384) (only column 329 is read twice). The
    boundary columns 109/219/274 that adaptive windows 2/4/5 need are
    patched into a pad column slot by the ACT engine from the previous
    span's resident slab instead of being re-read from HBM.
  - Span 4's H-chunk 0 streams f32 over the otherwise-idle sync HWDGE
    ring at block start (the SWDGE ring needs ~3us of init before its
    first descriptors) and DVE downcasts it; the other 14 slabs stream
    via SWDGE DMAs that cast f32 -> bf16 in flight into a 6-slot SBUF
    ring.
  - DMA descriptors deal round-robin over the 16 DMA engines with a
    persistent ring pointer (data descs + 16 completion descs each
    advance it, mod 16). Engine 15 is measurably ~23% slower than
    engines 0-14 (it carries system/profiling traffic), so each slab is
    emitted as a starve pattern: a 47-row transfer (47 = 2*16+15 gives
    engines 0-14 three rows, engine 15 two), a 1-descriptor 64B shim
    that lands on engine 15 and realigns the pointer, the bulk rows as
    a multiple-of-16 transfer, and the leftover row(s) as 16 equal
    column pieces. Engine 15 ends up with ~0.80 of a fair byte share,
    so all 16 engines finish streaming together instead of everyone
    throttling to engine 15's pace through the ring-WAR backpressure.
  - Span 3's slabs stream as two column pieces (each with its own
    starve pattern) so the tail window's matmuls pipeline with their
    arrival.
  - The pmat const loads via the ACT HWDGE ring.
  - TensorEngine reduces over H (the partition dim) with bf16 matmuls:
    stationary P_{j,k} (128 x 7) is a bf16 membership mask of the
    H-windows scaled by 1/(sh_i*sw_j), so no separate averaging pass is
    needed. Each window j owns ONE PSUM bank: all 21 matmuls (3 H-chunks
    x 7 moving-operand chunks of <=512 cols) accumulate into it, so the
    chunk dimension folds inside PSUM and windows never share banks --
    no PE<->DVE write-after-read chain between windows.
  - DVE does ONE op per window right after its stop-matmul: a strided
    X-reduce of the 512-col bank over w' straight into the output tile.
    The first six windows' results go out on sync right after window 4
    finishes; the tail window's 64 channels go out in a final small DMA.

Raw Bass blocks with explicit semaphores (TileContext's generated sync
exceeds this toolchain's per-instruction sync-wait limits).
"""

import numpy as np
import ml_dtypes

import concourse.bass as bass
import concourse.mybir as mybir
from concourse.bass_utils import run_bass_kernel_spmd

B, H, W, C = 8, 384, 384, 64
OUT = 7
N_CORES = 8
KH = H // 128  # 3 H-chunks of 128 rows
NCH = 7  # moving-operand chunks per window
# (first W column, data width, data offset inside the ring slot)
SPANS = [(0, 110, 0), (110, 110, 1), (220, 55, 1), (275, 55, 1), (329, 55, 0)]
STREAM = [4, 0, 1, 2, 3]  # span stream order (span 4 first: window 6 runs first)
NSW = 15  # slabs: 5 spans x 3 H-chunks (slab 0 goes via sync+DVE, not SWDGE)
SLOT = 111 * C  # ring slot size in elements
RING = 6  # slab ring depth
W512 = (512,) * 7
W448 = (512,) * 6 + (448,)
# (span g, view offset in slot cols, per-chunk widths)
WINDOWS = [
    (0, 0, W448),
    (0, 54, W512),
    (1, 0, W512),
    (1, 55, W512),
    (2, 0, W512),
    (3, 0, W512),
    (4, 0, W448),
]
FIRST_WIN = {4: 6, 0: 0, 1: 2, 2: 4, 3: 5}  # span -> its first window in WORDER
LAST_WIN = {4: 6, 0: 1, 1: 3, 2: 4, 3: 5}  # span -> its last window in WORDER
WORDER = [6, 0, 1, 2, 3, 4, 5]  # PE/DVE window processing order
# boundary-column patches: (src span, src slot col, dst span, WAR order idx)
PATCHES = [
    (0, 109, 1, 0),  # W109 for window 2: span0 col109 -> span1 pad, after w6
    (1, 110, 2, 1),  # W219 for window 4: span1 col110 -> span2 pad, after w0
    (2, 55, 3, 3),  # W274 for window 5: span2 col55 -> span3 pad, after w2
]
PAD_OF_WIN = {2: 0, 4: 1, 5: 2}  # window -> patch entry guarding its chunk 0

# starve-pattern row splits: k=1 -> [47, 80] + 1 leftover row;
# k=2 -> [47, 47, 32] + 2 leftover rows. Engine 15 loses k full rows.
STARVE_ROWS = {1: (47, 80), 2: (47, 47, 32)}
# per-span starve depth for whole slabs (span 3 uses column pieces, k=1 each)
SPAN_K = {4: 1, 0: 2, 1: 2, 2: 1}
TAIL_SPAN = 3
SPA = 4 * 512  # span-3 piece split point (slot view elems): chunks 0-3 | 4-6

_F32 = mybir.dt.float32
_BF16 = mybir.dt.bfloat16

SLAB_OF = {}
for _i in range(NSW):
    SLAB_OF[(STREAM[_i // KH], _i % KH)] = _i
SPAN_OF_SLAB = [STREAM[_i // KH] for _i in range(NSW)]


def _windows(d, out):
    starts = np.floor(np.arange(out) * d / out).astype(np.int64)
    ends = np.ceil((np.arange(out) + 1) * d / out).astype(np.int64)
    return starts, ends - starts


def _build():
    nc = bass.Bass(num_swdge_queues=1)
    x = nc.declare_dram_parameter("x", [H, W * C], _F32, isOutput=False)
    pmat = nc.declare_dram_parameter(
        "pmat", [128, OUT * KH * OUT], _BF16, isOutput=False
    )
    out = nc.declare_dram_parameter("out", [OUT, OUT * C], _F32, isOutput=True)

    # per-slab completion-semaphore targets, filled while emitting gpsimd DMAs
    sem_target = [0] * NSW
    semb_target = [0] * NSW

    with (
        nc.sbuf_tensor([128, RING * SLOT], _BF16) as xbuf,
        nc.sbuf_tensor([128, 55 * C], _F32) as stage,
        nc.sbuf_tensor([128, OUT * KH * OUT], _BF16) as p_sb,
        nc.sbuf_tensor([128, 16], _BF16) as shim_sb,
        nc.sbuf_tensor([OUT, OUT * C], _F32) as y_sb,
        nc.psum_tensor([128, OUT * 512], _F32) as psum,
        nc.semaphore("const_sem") as const_sem,
        nc.semaphore("warm_sem") as warm_sem,
        nc.semaphore("stage_sem") as stage_sem,
        nc.semaphore("pad_sem") as pad_sem,
        nc.semaphore("pe_pass_sem") as pe_pass_sem,
        nc.semaphore("win_sem") as win_sem,
        nc.semaphore("mul_sem") as mul_sem,
        nc.semaphore("out_sem") as out_sem,
    ):
        slab_sems = [nc.alloc_semaphore(f"slab{i}") for i in range(NSW)]
        pieceb_sems = [nc.alloc_semaphore(f"slab{i}b") for i in range(NSW)]

        ORDER_OF = {j: o for o, j in enumerate(WORDER)}

        def pass_wait(eng, o, k):
            """Wait until the PE finished pass (order-index o, H-chunk k)."""
            if k == KH - 1:
                eng.wait_ge(win_sem, o + 1)
            else:
                eng.wait_ge(pe_pass_sem, o * (KH - 1) + k + 1)

        def slot_col(i, col):
            return (i % RING) * SLOT + col * C

        with nc.Block(no_gpsimd_drain=True) as block:

            @block.gpsimd
            def _(gpsimd):
                ptr = [0]  # SWDGE ring deal pointer mod 16

                def shim(sem):
                    # 1 tiny desc that lands on engine (ptr) and advances it
                    gpsimd.dma_start(
                        out=shim_sb[0:1, 0:16], in_=x[0:1, 0:16]
                    ).then_inc(sem, 16)
                    ptr[0] = (ptr[0] + 1) % 16

                def emit_rows(i, k, sem, r0, nr, e0, ne, pdim):
                    """rows [r0, r0+nr) of slab i (H-chunk k), view elems
                    [e0, e0+ne) of the slot, as one transfer; pdim > 1
                    splits each row into pdim equal column pieces."""
                    g = SPAN_OF_SLAB[i]
                    w0, _, doff = SPANS[g]
                    c0 = w0 * C + (e0 - doff * C)
                    dst0 = slot_col(i, 0) + e0
                    o = xbuf[r0 : r0 + nr, dst0 : dst0 + ne]
                    s = x[k * 128 + r0 : k * 128 + r0 + nr, c0 : c0 + ne]
                    if pdim > 1:
                        o = o.rearrange("p (a b) -> p a b", a=pdim)
                        s = s.rearrange("p (a b) -> p a b", a=pdim)
                    gpsimd.dma_start(out=o, in_=s).then_inc(sem, 16)
                    ptr[0] = (ptr[0] + nr * pdim) % 16

                def emit_piece(i, k, sem, e0, ne, kst):
                    """all 128 rows of view elems [e0, e0+ne) with starve
                    depth kst; bumps the sem target by 16 per transfer."""
                    tgt = 0
                    assert ptr[0] == 0
                    r0 = 0
                    for nr in STARVE_ROWS[kst]:
                        emit_rows(i, k, sem, r0, nr, e0, ne, 1)
                        tgt += 16
                        r0 += nr
                        if ptr[0] == 15:
                            shim(sem)
                            tgt += 16
                    left = 128 - r0
                    if left:
                        emit_rows(i, k, sem, r0, left, e0, ne, 16 // left)
                        tgt += 16
                    assert ptr[0] == 0
                    return tgt

                for i in range(1, NSW):
                    g, k = SPAN_OF_SLAB[i], i % KH
                    _, wd, doff = SPANS[g]
                    if i >= RING:
                        gp, kp = SPAN_OF_SLAB[i - RING], (i - RING) % KH
                        pass_wait(gpsimd, ORDER_OF[LAST_WIN[gp]], kp)
                    if g == TAIL_SPAN:
                        sem_target[i] = emit_piece(
                            i, k, slab_sems[i], doff * C, SPA - doff * C, 1
                        )
                        semb_target[i] = emit_piece(
                            i, k, pieceb_sems[i], SPA, (doff + wd) * C - SPA, 1
                        )
                    else:
                        sem_target[i] = emit_piece(
                            i, k, slab_sems[i], doff * C, wd * C, SPAN_K[g]
                        )

            @block.sync
            def _(sync):
                # span 4 H-chunk 0 as f32 while the SWDGE ring initializes
                w0, wd, _ = SPANS[4]
                sync.dma_start(
                    out=stage[:], in_=x[0:128, w0 * C : (w0 + wd) * C]
                ).then_inc(stage_sem, 16)
                # windows 0-4 and 6 are final once window 4 (order 5) scaled
                sync.wait_ge(mul_sem, 1)
                sync.dma_start(
                    out=out[:, 0 : 5 * C], in_=y_sb[:, 0 : 5 * C]
                ).then_inc(out_sem, 16)
                sync.dma_start(
                    out=out[:, 6 * C : 7 * C], in_=y_sb[:, 6 * C : 7 * C]
                ).then_inc(out_sem, 16)
                # tail window 5
                sync.wait_ge(mul_sem, 2)
                sync.dma_start(
                    out=out[:, 5 * C : 6 * C], in_=y_sb[:, 5 * C : 6 * C]
                ).then_inc(out_sem, 16)
                sync.wait_ge(out_sem, 48)

            @block.scalar
            def _(scalar):
                scalar.dma_start(out=p_sb[:], in_=pmat[:]).then_inc(const_sem, 16)
                # patch the boundary columns into the pad slots (64 bf16
                # elems x 128 partitions each)
                for ent, (src_g, src_col, dst_g, war_o) in enumerate(PATCHES):
                    for k in range(KH):
                        si = SLAB_OF[(src_g, k)]
                        di = SLAB_OF[(dst_g, k)]
                        if si == 0:
                            scalar.wait_ge(warm_sem, 1)
                        else:
                            scalar.wait_ge(slab_sems[si], sem_target[si])
                        # WAR: the pad column slot still holds data the
                        # previous occupant's windows read
                        pass_wait(scalar, war_o, k)
                        scalar.copy(
                            xbuf[:, slot_col(di, 0) : slot_col(di, 1)],
                            xbuf[
                                :,
                                slot_col(si, src_col) : slot_col(
                                    si, src_col + 1
                                ),
                            ],
                        ).then_inc(pad_sem, 1)

            @block.tensor
            def _(tensor):
                tensor.wait_ge(const_sem, 16)
                for o, j in enumerate(WORDER):
                    g, off, widths = WINDOWS[j]
                    for k in range(KH):
                        i = SLAB_OF[(g, k)]
                        if j == FIRST_WIN[g]:
                            if i == 0:
                                tensor.wait_ge(warm_sem, 1)
                            else:
                                tensor.wait_ge(slab_sems[i], sem_target[i])
                        if j in PAD_OF_WIN:
                            tensor.wait_ge(
                                pad_sem, PAD_OF_WIN[j] * KH + k + 1
                            )
                        base = slot_col(i, off)
                        n = j * KH + k
                        lhsT = p_sb[:, n * OUT : (n + 1) * OUT]
                        for cb in range(NCH):
                            if g == TAIL_SPAN and cb == 4:
                                tensor.wait_ge(
                                    pieceb_sems[i], semb_target[i]
                                )
                            mm = tensor.matmul(
                                psum[:OUT, j * 512 : j * 512 + widths[cb]],
                                lhsT,
                                xbuf[
                                    :,
                                    base + cb * 512 : base
                                    + cb * 512
                                    + widths[cb],
                                ],
                                start=(k == 0 and cb == 0),
                                stop=(k == KH - 1 and cb == NCH - 1),
                            )
                        if k == KH - 1:
                            mm.then_inc(win_sem, 1)
                        else:
                            mm.then_inc(pe_pass_sem, 1)

            @block.vector
            def _(vector):
                # downcast the sync-streamed span4 H-chunk 0 into ring slot 0
                vector.wait_ge(stage_sem, 16)
                vector.tensor_copy(xbuf[:, 0 : 55 * C], stage[:]).then_inc(
                    warm_sem, 1
                )
                for o, j in enumerate(WORDER):
                    vector.wait_ge(win_sem, o + 1)
                    # fold the bank's 8 w' column groups into the window
                    # average (the 1/(sh*sw) scale is baked into pmat)
                    red = vector.tensor_reduce(
                        out=y_sb[:, j * C : (j + 1) * C],
                        in_=psum[:OUT, j * 512 : (j + 1) * 512].rearrange(
                            "p (w c) -> p c w", c=C
                        ),
                        axis=mybir.AxisListType.X,
                        op=mybir.AluOpType.add,
                    )
                    if o >= OUT - 2:
                        red.then_inc(mul_sem, 1)

    return nc


def _consts():
    hs, hsz = _windows(H, OUT)
    _, wsz = _windows(W, OUT)
    p = np.zeros((128, OUT * KH * OUT), np.float32)
    for j in range(OUT):
        for k in range(KH):
            n = j * KH + k
            for i in range(OUT):
                h0, h1 = int(hs[i]), int(hs[i] + hsz[i])
                for h in range(max(h0, k * 128), min(h1, (k + 1) * 128)):
                    p[h - k * 128, n * OUT + i] = 1.0 / (
                        float(hsz[i]) * float(wsz[j])
                    )
    return p.astype(ml_dtypes.bfloat16)


_NC_CACHE = None


def _run(x, **kwargs):
    global _NC_CACHE
    if _NC_CACHE is None:
        _NC_CACHE = _build()
    nc = _NC_CACHE
    p = _consts()
    x = np.ascontiguousarray(np.asarray(x, dtype=np.float32))
    in_maps = [
        {"x": x[b].reshape(H, W * C), "pmat": p}
        for b in range(N_CORES)
    ]
    res = run_bass_kernel_spmd(nc, in_maps, core_ids=list(range(N_CORES)), **kwargs)
    y = np.stack(
        [res.results[b]["out"].reshape(OUT, OUT, C) for b in range(N_CORES)]
    )
    return y, res


def kernel(x: np.ndarray) -> np.ndarray:
    y, _ = _run(x)
    return y


# revision 6
# speedup vs baseline: 1.4533x; 1.4533x over previous
"""Adaptive average pooling (8,384,384,64) NHWC -> (8,7,7,64) on 8 TRN2 NeuronCores.

Pure data parallel: one batch sample per core, no collectives. Per core:
  - W is covered by 5 nearly non-overlapping spans [0,110) [110,220)
    [220,275) [275,330) [329,384) (only column 329 is read twice). The
    boundary columns 109/219/274 that adaptive windows 2/4/5 need are
    patched into a pad column slot by the ACT engine from the previous
    span's resident slab instead of being re-read from HBM.
  - Span 4's H-chunk 0 streams f32 over the otherwise-idle sync HWDGE
    ring at block start (the SWDGE ring needs ~3us of init before its
    first descriptors) and DVE downcasts it; the other 14 slabs stream
    via SWDGE DMAs that cast f32 -> bf16 in flight into a 6-slot SBUF
    ring.
  - DMA descriptors deal round-robin over the 16 DMA engines with a
    persistent ring pointer (data descs + 16 completion descs each
    advance it, mod 16). Engine 15 is measurably ~23% slower than
    engines 0-14 (it carries system/profiling traffic), so each slab is
    emitted as a starve pattern: a 47-row transfer (47 = 2*16+15 gives
    engines 0-14 three rows, engine 15 two), a 1-descriptor 64B shim
    that lands on engine 15 and realigns the pointer, the bulk rows as
    a multiple-of-16 transfer, and the leftover row(s) as 16 equal
    column pieces. Engine 15 ends up with ~0.80 of a fair byte share,
    so all 16 engines finish streaming together instead of everyone
    throttling to engine 15's pace through the ring-WAR backpressure.
  - Span 3's slabs stream as two column pieces (each with its own
    starve pattern) so the tail window's matmuls pipeline with their
    arrival.
  - The pmat const loads via the ACT HWDGE ring.
  - TensorEngine reduces over H (the partition dim) with bf16 matmuls:
    stationary P_{j,k} (128 x 7) is a bf16 membership mask of the
    H-windows scaled by 1/(sh_i*sw_j), so no separate averaging pass is
    needed. Each window j owns ONE PSUM bank: all 21 matmuls (3 H-chunks
    x 7 moving-operand chunks of <=512 cols) accumulate into it, so the
    chunk dimension folds inside PSUM and windows never share banks --
    no PE<->DVE write-after-read chain between windows.
  - DVE does ONE op per window right after its stop-matmul: a strided
    X-reduce of the 512-col bank over w' straight into the output tile.
    The first six windows' results go out on sync right after window 4
    finishes; the tail window's 64 channels go out in a final small DMA.

Raw Bass blocks with explicit semaphores (TileContext's generated sync
exceeds this toolchain's per-instruction sync-wait limits).
"""

import numpy as np
import ml_dtypes

import concourse.bass as bass
import concourse.mybir as mybir
from concourse.bass_utils import run_bass_kernel_spmd

B, H, W, C = 8, 384, 384, 64
OUT = 7
N_CORES = 8
KH = H // 128  # 3 H-chunks of 128 rows
NCH = 7  # moving-operand chunks per window
# (first W column, data width, data offset inside the ring slot)
SPANS = [(0, 110, 0), (110, 110, 1), (220, 55, 1), (275, 55, 1), (329, 55, 0)]
STREAM = [4, 0, 1, 2, 3]  # span stream order (span 4 first: window 6 runs first)
NSW = 15  # slabs: 5 spans x 3 H-chunks (slab 0 goes via sync+DVE, not SWDGE)
SLOT = 111 * C  # ring slot size in elements
RING = 6  # slab ring depth
W512 = (512,) * 7
W448 = (512,) * 6 + (448,)
# (span g, view offset in slot cols, per-chunk widths)
WINDOWS = [
    (0, 0, W448),
    (0, 54, W512),
    (1, 0, W512),
    (1, 55, W512),
    (2, 0, W512),
    (3, 0, W512),
    (4, 0, W448),
]
FIRST_WIN = {4: 6, 0: 0, 1: 2, 2: 4, 3: 5}  # span -> its first window in WORDER
LAST_WIN = {4: 6, 0: 1, 1: 3, 2: 4, 3: 5}  # span -> its last window in WORDER
WORDER = [6, 0, 1, 2, 3, 4, 5]  # PE/DVE window processing order
# boundary-column patches: (src span, src slot col, dst span, WAR order idx)
PATCHES = [
    (0, 109, 1, 0),  # W109 for window 2: span0 col109 -> span1 pad, after w6
    (1, 110, 2, 1),  # W219 for window 4: span1 col110 -> span2 pad, after w0
    (2, 55, 3, 3),  # W274 for window 5: span2 col55 -> span3 pad, after w2
]
PAD_OF_WIN = {2: 0, 4: 1, 5: 2}  # window -> patch entry guarding its chunk 0

# starve-pattern row splits: each (runt, nice) pair is [14-row runt to
# engines 0-13][2-row shim to engines 14,15][nice rows uniform]; the
# final 2*k rows go as 16 equal column-piece units (uniform). Engines
# 14 and 15 each lose k full rows per slab.
STARVE_ROWS = {1: ((14, 112),), 2: ((14, 0), (14, 96))}
# per-span starve depth for whole slabs (span 3 uses column pieces, k=1 each)
SPAN_K = {4: 1, 0: 2, 1: 2, 2: 1}
TAIL_SPAN = 3
SPA = 4 * 512  # span-3 piece split point (slot view elems): chunks 0-3 | 4-6

_F32 = mybir.dt.float32
_BF16 = mybir.dt.bfloat16

SLAB_OF = {}
for _i in range(NSW):
    SLAB_OF[(STREAM[_i // KH], _i % KH)] = _i
SPAN_OF_SLAB = [STREAM[_i // KH] for _i in range(NSW)]


def _windows(d, out):
    starts = np.floor(np.arange(out) * d / out).astype(np.int64)
    ends = np.ceil((np.arange(out) + 1) * d / out).astype(np.int64)
    return starts, ends - starts


def _build():
    nc = bass.Bass(num_swdge_queues=2)
    x = nc.declare_dram_parameter("x", [H, W * C], _F32, isOutput=False)
    pmat = nc.declare_dram_parameter(
        "pmat", [128, OUT * KH * OUT], _BF16, isOutput=False
    )
    out = nc.declare_dram_parameter("out", [OUT, OUT * C], _F32, isOutput=True)

    # per-slab completion-semaphore targets, filled while emitting gpsimd DMAs
    sem_target = [0] * NSW
    semb_target = [0] * NSW

    with (
        nc.sbuf_tensor([128, RING * SLOT], _BF16) as xbuf,
        nc.sbuf_tensor([128, 55 * C], _F32) as stage,
        nc.sbuf_tensor([128, OUT * KH * OUT], _BF16) as p_sb,
        nc.sbuf_tensor([128, 16], _BF16) as shim_sb,
        nc.sbuf_tensor([OUT, OUT * C], _F32) as y_sb,
        nc.psum_tensor([128, OUT * 512], _F32) as psum,
        nc.semaphore("const_sem") as const_sem,
        nc.semaphore("warm_sem") as warm_sem,
        nc.semaphore("stage_sem") as stage_sem,
        nc.semaphore("pad_sem") as pad_sem,
        nc.semaphore("pe_pass_sem") as pe_pass_sem,
        nc.semaphore("win_sem") as win_sem,
        nc.semaphore("mul_sem") as mul_sem,
        nc.semaphore("out_sem") as out_sem,
    ):
        slab_sems = [nc.alloc_semaphore(f"slab{i}") for i in range(NSW)]
        pieceb_sems = [nc.alloc_semaphore(f"slab{i}b") for i in range(NSW)]

        ORDER_OF = {j: o for o, j in enumerate(WORDER)}

        def pass_wait(eng, o, k):
            """Wait until the PE finished pass (order-index o, H-chunk k)."""
            if k == KH - 1:
                eng.wait_ge(win_sem, o + 1)
            else:
                eng.wait_ge(pe_pass_sem, o * (KH - 1) + k + 1)

        def slot_col(i, col):
            return (i % RING) * SLOT + col * C

        with nc.Block(no_gpsimd_drain=True) as block:

            @block.gpsimd
            def _(gpsimd):

                def shim(sem):
                    # 2-row runt: tiny descs to engines 14, 15; realigns the
                    # deal pointer to 0 after a 14-row runt
                    gpsimd.dma_start(
                        out=shim_sb[0:2, 0:8], in_=x[0:2, 0:8]
                    ).then_inc(sem, 16)

                def emit_rows(i, k, sem, r0, nr, e0, ne, pdim):
                    """rows [r0, r0+nr) of slab i (H-chunk k), view elems
                    [e0, e0+ne) of the slot, as one transfer; pdim > 1
                    splits each row into pdim equal column pieces."""
                    g = SPAN_OF_SLAB[i]
                    w0, _, doff = SPANS[g]
                    c0 = w0 * C + (e0 - doff * C)
                    dst0 = slot_col(i, 0) + e0
                    o = xbuf[r0 : r0 + nr, dst0 : dst0 + ne]
                    s = x[k * 128 + r0 : k * 128 + r0 + nr, c0 : c0 + ne]
                    if pdim > 1:
                        o = o.rearrange("p (a b) -> p a b", a=pdim)
                        s = s.rearrange("p (a b) -> p a b", a=pdim)
                    gpsimd.dma_start(out=o, in_=s).then_inc(sem, 16)

                def emit_piece(i, k, sem, e0, ne, kst):
                    """all 128 rows of view elems [e0, e0+ne) with starve
                    depth kst; bumps the sem target by 16 per transfer."""
                    tgt = 0
                    r0 = 0
                    for runt, nice in STARVE_ROWS[kst]:
                        emit_rows(i, k, sem, r0, runt, e0, ne, 1)
                        shim(sem)
                        tgt += 32
                        r0 += runt
                        if nice:
                            emit_rows(i, k, sem, r0, nice, e0, ne, 1)
                            tgt += 16
                            r0 += nice
                    left = 128 - r0
                    if left:
                        emit_rows(i, k, sem, r0, left, e0, ne, 16 // left)
                        tgt += 16
                    return tgt

                for i in range(1, NSW):
                    g, k = SPAN_OF_SLAB[i], i % KH
                    _, wd, doff = SPANS[g]
                    if i >= RING:
                        gp, kp = SPAN_OF_SLAB[i - RING], (i - RING) % KH
                        pass_wait(gpsimd, ORDER_OF[LAST_WIN[gp]], kp)
                    if g == TAIL_SPAN:
                        sem_target[i] = emit_piece(
                            i, k, slab_sems[i], doff * C, SPA - doff * C, 1
                        )
                        semb_target[i] = emit_piece(
                            i, k, pieceb_sems[i], SPA, (doff + wd) * C - SPA, 1
                        )
                    else:
                        sem_target[i] = emit_piece(
                            i, k, slab_sems[i], doff * C, wd * C, SPAN_K[g]
                        )

            @block.sync
            def _(sync):
                # span 4 H-chunk 0 as f32 while the SWDGE ring initializes
                w0, wd, _ = SPANS[4]
                sync.dma_start(
                    out=stage[:], in_=x[0:128, w0 * C : (w0 + wd) * C]
                ).then_inc(stage_sem, 16)
                # windows 0-4 and 6 are final once window 4 (order 5) scaled
                sync.wait_ge(mul_sem, 1)
                sync.dma_start(
                    out=out[:, 0 : 5 * C], in_=y_sb[:, 0 : 5 * C]
                ).then_inc(out_sem, 16)
                sync.dma_start(
                    out=out[:, 6 * C : 7 * C], in_=y_sb[:, 6 * C : 7 * C]
                ).then_inc(out_sem, 16)
                # tail window 5
                sync.wait_ge(mul_sem, 2)
                sync.dma_start(
                    out=out[:, 5 * C : 6 * C], in_=y_sb[:, 5 * C : 6 * C]
                ).then_inc(out_sem, 16)
                sync.wait_ge(out_sem, 48)

            @block.scalar
            def _(scalar):
                scalar.dma_start(out=p_sb[:], in_=pmat[:]).then_inc(const_sem, 16)
                # patch the boundary columns into the pad slots (64 bf16
                # elems x 128 partitions each)
                for ent, (src_g, src_col, dst_g, war_o) in enumerate(PATCHES):
                    for k in range(KH):
                        si = SLAB_OF[(src_g, k)]
                        di = SLAB_OF[(dst_g, k)]
                        if si == 0:
                            scalar.wait_ge(warm_sem, 1)
                        else:
                            scalar.wait_ge(slab_sems[si], sem_target[si])
                        # WAR: the pad column slot still holds data the
                        # previous occupant's windows read
                        pass_wait(scalar, war_o, k)
                        scalar.copy(
                            xbuf[:, slot_col(di, 0) : slot_col(di, 1)],
                            xbuf[
                                :,
                                slot_col(si, src_col) : slot_col(
                                    si, src_col + 1
                                ),
                            ],
                        ).then_inc(pad_sem, 1)

            @block.tensor
            def _(tensor):
                tensor.wait_ge(const_sem, 16)
                for o, j in enumerate(WORDER):
                    g, off, widths = WINDOWS[j]
                    for k in range(KH):
                        i = SLAB_OF[(g, k)]
                        if j == FIRST_WIN[g]:
                            if i == 0:
                                tensor.wait_ge(warm_sem, 1)
                            else:
                                tensor.wait_ge(slab_sems[i], sem_target[i])
                        if j in PAD_OF_WIN:
                            tensor.wait_ge(
                                pad_sem, PAD_OF_WIN[j] * KH + k + 1
                            )
                        base = slot_col(i, off)
                        n = j * KH + k
                        lhsT = p_sb[:, n * OUT : (n + 1) * OUT]
                        for cb in range(NCH):
                            if g == TAIL_SPAN and cb == 4:
                                tensor.wait_ge(
                                    pieceb_sems[i], semb_target[i]
                                )
                            mm = tensor.matmul(
                                psum[:OUT, j * 512 : j * 512 + widths[cb]],
                                lhsT,
                                xbuf[
                                    :,
                                    base + cb * 512 : base
                                    + cb * 512
                                    + widths[cb],
                                ],
                                start=(k == 0 and cb == 0),
                                stop=(k == KH - 1 and cb == NCH - 1),
                            )
                        if k == KH - 1:
                            mm.then_inc(win_sem, 1)
                        else:
                            mm.then_inc(pe_pass_sem, 1)

            @block.vector
            def _(vector):
                # downcast the sync-streamed span4 H-chunk 0 into ring slot 0
                vector.wait_ge(stage_sem, 16)
                vector.tensor_copy(xbuf[:, 0 : 55 * C], stage[:]).then_inc(
                    warm_sem, 1
                )
                for o, j in enumerate(WORDER):
                    vector.wait_ge(win_sem, o + 1)
                    # fold the bank's 8 w' column groups into the window
                    # average (the 1/(sh*sw) scale is baked into pmat)
                    red = vector.tensor_reduce(
                        out=y_sb[:, j * C : (j + 1) * C],
                        in_=psum[:OUT, j * 512 : (j + 1) * 512].rearrange(
                            "p (w c) -> p c w", c=C
                        ),
                        axis=mybir.AxisListType.X,
                        op=mybir.AluOpType.add,
                    )
                    if o >= OUT - 2:
                        red.then_inc(mul_sem, 1)

    return nc


def _consts():
    hs, hsz = _windows(H, OUT)
    _, wsz = _windows(W, OUT)
    p = np.zeros((128, OUT * KH * OUT), np.float32)
    for j in range(OUT):
        for k in range(KH):
            n = j * KH + k
            for i in range(OUT):
                h0, h1 = int(hs[i]), int(hs[i] + hsz[i])
                for h in range(max(h0, k * 128), min(h1, (k + 1) * 128)):
                    p[h - k * 128, n * OUT + i] = 1.0 / (
                        float(hsz[i]) * float(wsz[j])
                    )
    return p.astype(ml_dtypes.bfloat16)


_NC_CACHE = None


def _run(x, **kwargs):
    global _NC_CACHE
    if _NC_CACHE is None:
        _NC_CACHE = _build()
    nc = _NC_CACHE
    p = _consts()
    x = np.ascontiguousarray(np.asarray(x, dtype=np.float32))
    in_maps = [
        {"x": x[b].reshape(H, W * C), "pmat": p}
        for b in range(N_CORES)
    ]
    res = run_bass_kernel_spmd(nc, in_maps, core_ids=list(range(N_CORES)), **kwargs)
    y = np.stack(
        [res.results[b]["out"].reshape(OUT, OUT, C) for b in range(N_CORES)]
    )
    return y, res


def kernel(x: np.ndarray) -> np.ndarray:
    y, _ = _run(x)
    return y


# revision 7
# speedup vs baseline: 1.5023x; 1.0337x over previous
"""Adaptive average pooling (8,384,384,64) NHWC -> (8,7,7,64) on 8 TRN2 NeuronCores.

Pure data parallel: one batch sample per core, no collectives. Per core:
  - W is covered by 5 nearly non-overlapping spans [0,110) [110,220)
    [220,275) [275,330) [329,384) (only column 329 is read twice). The
    boundary columns 109/219/274 that adaptive windows 2/4/5 need are
    patched into a pad column slot by the ACT engine from the previous
    span's resident slab instead of being re-read from HBM.
  - Span 4's H-chunk 0 streams f32 over the otherwise-idle sync HWDGE
    ring at block start (the SWDGE Q7 needs ~3us of init before its
    first descriptors) and DVE downcasts it; the other 14 slabs stream
    via SWDGE DMAs that cast f32 -> bf16 in flight, alternating two
    SWDGE queues over a 6-slot SBUF ring. Span 3's H-chunk 0 and 2
    slabs each stream as two pieces so the tail window's matmuls
    pipeline with their arrival.
  - SDMA engines 14/15 run ~20% slower than engines 0-13 under SWDGE
    load (descriptor-ring AXI port contention). The SWDGE deal is
    deterministic: descriptors deal per ROW round-robin over the 16
    engines with a persistent pointer; a transfer of R<16 rows covers
    engines P..P+R-1 and advances P by R; R%16==0 transfers are
    uniform and pointer-neutral. Two 110-col slabs are therefore
    emitted as full-starve patterns -- 8x([14-row runt to engines
    0-13] + [2-row 64B shim to engines 14,15]) + [16-row uniform] --
    which takes engines 14/15 down to ~0.82 of a fair byte share so
    all 16 engines finish streaming together instead of the whole
    pipeline throttling to engine 15's pace via ring-WAR backpressure.
  - The pmat const loads via the ACT HWDGE ring.
  - TensorEngine reduces over H (the partition dim) with bf16 matmuls:
    stationary P_{j,k} (128 x 7) is a bf16 membership mask of the
    H-windows scaled by 1/(sh_i*sw_j), so no separate averaging pass is
    needed. Each window j owns ONE PSUM bank: all 21 matmuls (3 H-chunks
    x 7 moving-operand chunks of <=512 cols) accumulate into it, so the
    chunk dimension folds inside PSUM and windows never share banks --
    no PE<->DVE write-after-read chain between windows.
  - DVE does ONE op per window right after its stop-matmul: a strided
    X-reduce of the 512-col bank over w' straight into the output tile.
    The first six windows' results go out on sync right after window 4
    finishes; the tail window's 64 channels go out in a final small DMA.

Raw Bass blocks with explicit semaphores (TileContext's generated sync
exceeds this toolchain's per-instruction sync-wait limits).
"""

import numpy as np
import ml_dtypes

import concourse.bass as bass
import concourse.mybir as mybir
from concourse.bass_utils import run_bass_kernel_spmd

B, H, W, C = 8, 384, 384, 64
OUT = 7
N_CORES = 8
KH = H // 128  # 3 H-chunks of 128 rows
NCH = 7  # moving-operand chunks per window
# (first W column, data width, data offset inside the ring slot)
SPANS = [(0, 110, 0), (110, 110, 1), (220, 55, 1), (275, 55, 1), (329, 55, 0)]
STREAM = [4, 0, 1, 2, 3]  # span stream order (span 4 first: window 6 runs first)
NSW = 15  # slabs: 5 spans x 3 H-chunks (slab 0 goes via sync+DVE, not SWDGE)
SLOT = 111 * C  # ring slot size in elements
RING = 6  # slab ring depth
W512 = (512,) * 7
W448 = (512,) * 6 + (448,)
# (span g, view offset in slot cols, per-chunk widths)
WINDOWS = [
    (0, 0, W448),
    (0, 54, W512),
    (1, 0, W512),
    (1, 55, W512),
    (2, 0, W512),
    (3, 0, W512),
    (4, 0, W448),
]
FIRST_WIN = {4: 6, 0: 0, 1: 2, 2: 4, 3: 5}  # span -> its first window in WORDER
LAST_WIN = {4: 6, 0: 1, 1: 3, 2: 4, 3: 5}  # span -> its last window in WORDER
WORDER = [6, 0, 1, 2, 3, 4, 5]  # PE/DVE window processing order
# boundary-column patches: (src span, src slot col, dst span, WAR order idx)
PATCHES = [
    (0, 109, 1, 0),  # W109 for window 2: span0 col109 -> span1 pad, after w6
    (1, 110, 2, 1),  # W219 for window 4: span1 col110 -> span2 pad, after w0
    (2, 55, 3, 3),  # W274 for window 5: span2 col55 -> span3 pad, after w2
]
PAD_OF_WIN = {2: 0, 4: 1, 5: 2}  # window -> patch entry guarding its chunk 0

STARVE = (4, 7)  # span0 k1, span1 k1: full-starve slabs (engines 14/15)

_F32 = mybir.dt.float32
_BF16 = mybir.dt.bfloat16

SLAB_OF = {}
for _i in range(NSW):
    SLAB_OF[(STREAM[_i // KH], _i % KH)] = _i
SPAN_OF_SLAB = [STREAM[_i // KH] for _i in range(NSW)]
SPLIT = (12, 14)  # span-3 k0/k2 slabs stream as two pieces
SPA = 4 * 512  # piece split point (view elems): chunks 0-3 | 4-6


def _windows(d, out):
    starts = np.floor(np.arange(out) * d / out).astype(np.int64)
    ends = np.ceil((np.arange(out) + 1) * d / out).astype(np.int64)
    return starts, ends - starts


def _build():
    nc = bass.Bass(num_swdge_queues=2)
    x = nc.declare_dram_parameter("x", [H, W * C], _F32, isOutput=False)
    pmat = nc.declare_dram_parameter(
        "pmat", [128, OUT * KH * OUT], _BF16, isOutput=False
    )
    out = nc.declare_dram_parameter("out", [OUT, OUT * C], _F32, isOutput=True)

    sem_target = [16] * NSW  # per-slab completion-sem targets

    with (
        nc.sbuf_tensor([128, RING * SLOT], _BF16) as xbuf,
        nc.sbuf_tensor([128, 55 * C], _F32) as stage,
        nc.sbuf_tensor([128, OUT * KH * OUT], _BF16) as p_sb,
        nc.sbuf_tensor([128, 16], _BF16) as shim_sb,
        nc.sbuf_tensor([OUT, OUT * C], _F32) as y_sb,
        nc.psum_tensor([128, OUT * 512], _F32) as psum,
        nc.semaphore("const_sem") as const_sem,
        nc.semaphore("warm_sem") as warm_sem,
        nc.semaphore("stage_sem") as stage_sem,
        nc.semaphore("pad_sem") as pad_sem,
        nc.semaphore("pe_pass_sem") as pe_pass_sem,
        nc.semaphore("win_sem") as win_sem,
        nc.semaphore("mul_sem") as mul_sem,
        nc.semaphore("out_sem") as out_sem,
    ):
        slab_sems = [nc.alloc_semaphore(f"slab{i}") for i in range(NSW)]
        pieceb_sems = {i: nc.alloc_semaphore(f"slab{i}b") for i in SPLIT}

        ORDER_OF = {j: o for o, j in enumerate(WORDER)}

        def pass_wait(eng, o, k):
            """Wait until the PE finished pass (order-index o, H-chunk k)."""
            if k == KH - 1:
                eng.wait_ge(win_sem, o + 1)
            else:
                eng.wait_ge(pe_pass_sem, o * (KH - 1) + k + 1)

        def slot_col(i, col):
            return (i % RING) * SLOT + col * C

        with nc.Block(no_gpsimd_drain=True) as block:

            @block.gpsimd
            def _(gpsimd):
                def starve_slab(i, k, w0, wd, doff):
                    """128 rows as 8x(14-runt + 2-shim) + 16-nice: engines
                    0-13 carry 9 rows each, engines 14/15 one row + tiny
                    shims. Pointer-neutral (advance = 8*16 + 16 = 0 mod 16).
                    """
                    for j in range(8):
                        r0 = 14 * j
                        gpsimd.dma_start(
                            out=xbuf[
                                r0 : r0 + 14,
                                slot_col(i, doff) : slot_col(i, doff + wd),
                            ],
                            in_=x[
                                k * 128 + r0 : k * 128 + r0 + 14,
                                w0 * C : (w0 + wd) * C,
                            ],
                        ).then_inc(slab_sems[i], 16)
                        gpsimd.dma_start(
                            out=shim_sb[0:2, 0:8], in_=x[0:2, 0:8]
                        ).then_inc(slab_sems[i], 16)
                    gpsimd.dma_start(
                        out=xbuf[
                            112:128,
                            slot_col(i, doff) : slot_col(i, doff + wd),
                        ],
                        in_=x[k * 128 + 112 : (k + 1) * 128, w0 * C : (w0 + wd) * C],
                    ).then_inc(slab_sems[i], 16)
                    sem_target[i] = 17 * 16

                for i in range(1, NSW):
                    g, k = SPAN_OF_SLAB[i], i % KH
                    w0, wd, doff = SPANS[g]
                    if i >= RING:
                        gp, kp = SPAN_OF_SLAB[i - RING], (i - RING) % KH
                        pass_wait(gpsimd, ORDER_OF[LAST_WIN[gp]], kp)
                    if i in STARVE:
                        starve_slab(i, k, w0, wd, doff)
                        continue
                    if i in SPLIT:
                        gpsimd.dma_start(
                            out=xbuf[
                                :, slot_col(i, doff) : slot_col(i, 0) + SPA
                            ],
                            in_=x[
                                k * 128 : (k + 1) * 128,
                                w0 * C : w0 * C + SPA - doff * C,
                            ],
                        ).then_inc(slab_sems[i], 16)
                        gpsimd.dma_start(
                            out=xbuf[
                                :,
                                slot_col(i, 0) + SPA : slot_col(i, doff)
                                + wd * C,
                            ],
                            in_=x[
                                k * 128 : (k + 1) * 128,
                                w0 * C + SPA - doff * C : (w0 + wd) * C,
                            ],
                        ).then_inc(pieceb_sems[i], 16)
                        continue
                    dma = gpsimd.dma_start(
                        out=xbuf[:, slot_col(i, doff) : slot_col(i, doff + wd)],
                        in_=x[k * 128 : (k + 1) * 128, w0 * C : (w0 + wd) * C],
                    ).then_inc(slab_sems[i], 16)
                    if i % 2 == 1:
                        dma.ins.queue = "qPoolDynamic1"

            @block.sync
            def _(sync):
                # span 4 H-chunk 0 as f32 while the SWDGE Q7 initializes
                w0, wd, _ = SPANS[4]
                sync.dma_start(
                    out=stage[:], in_=x[0:128, w0 * C : (w0 + wd) * C]
                ).then_inc(stage_sem, 16)
                # windows 0-4 and 6 are final once window 4 (order 5) scaled
                sync.wait_ge(mul_sem, 1)
                sync.dma_start(
                    out=out[:, 0 : 5 * C], in_=y_sb[:, 0 : 5 * C]
                ).then_inc(out_sem, 16)
                sync.dma_start(
                    out=out[:, 6 * C : 7 * C], in_=y_sb[:, 6 * C : 7 * C]
                ).then_inc(out_sem, 16)
                # tail window 5
                sync.wait_ge(mul_sem, 2)
                sync.dma_start(
                    out=out[:, 5 * C : 6 * C], in_=y_sb[:, 5 * C : 6 * C]
                ).then_inc(out_sem, 16)
                sync.wait_ge(out_sem, 48)

            @block.scalar
            def _(scalar):
                scalar.dma_start(out=p_sb[:], in_=pmat[:]).then_inc(const_sem, 16)
                # patch the boundary columns into the pad slots (64 bf16
                # elems x 128 partitions each)
                for ent, (src_g, src_col, dst_g, war_o) in enumerate(PATCHES):
                    for k in range(KH):
                        si = SLAB_OF[(src_g, k)]
                        di = SLAB_OF[(dst_g, k)]
                        if si == 0:
                            scalar.wait_ge(warm_sem, 1)
                        else:
                            scalar.wait_ge(slab_sems[si], sem_target[si])
                        # WAR: the pad column slot still holds data the
                        # previous occupant's windows read
                        pass_wait(scalar, war_o, k)
                        scalar.copy(
                            xbuf[:, slot_col(di, 0) : slot_col(di, 1)],
                            xbuf[
                                :,
                                slot_col(si, src_col) : slot_col(
                                    si, src_col + 1
                                ),
                            ],
                        ).then_inc(pad_sem, 1)

            @block.tensor
            def _(tensor):
                tensor.wait_ge(const_sem, 16)
                for o, j in enumerate(WORDER):
                    g, off, widths = WINDOWS[j]
                    for k in range(KH):
                        i = SLAB_OF[(g, k)]
                        if j == FIRST_WIN[g]:
                            if i == 0:
                                tensor.wait_ge(warm_sem, 1)
                            else:
                                tensor.wait_ge(slab_sems[i], sem_target[i])
                        if j in PAD_OF_WIN:
                            tensor.wait_ge(
                                pad_sem, PAD_OF_WIN[j] * KH + k + 1
                            )
                        base = slot_col(i, off)
                        n = j * KH + k
                        lhsT = p_sb[:, n * OUT : (n + 1) * OUT]
                        for cb in range(NCH):
                            if i in SPLIT and cb == 4:
                                tensor.wait_ge(pieceb_sems[i], 16)
                            mm = tensor.matmul(
                                psum[:OUT, j * 512 : j * 512 + widths[cb]],
                                lhsT,
                                xbuf[
                                    :,
                                    base + cb * 512 : base
                                    + cb * 512
                                    + widths[cb],
                                ],
                                start=(k == 0 and cb == 0),
                                stop=(k == KH - 1 and cb == NCH - 1),
                            )
                        if k == KH - 1:
                            mm.then_inc(win_sem, 1)
                        else:
                            mm.then_inc(pe_pass_sem, 1)

            @block.vector
            def _(vector):
                # downcast the sync-streamed span4 H-chunk 0 into ring slot 0
                vector.wait_ge(stage_sem, 16)
                vector.tensor_copy(xbuf[:, 0 : 55 * C], stage[:]).then_inc(
                    warm_sem, 1
                )
                for o, j in enumerate(WORDER):
                    vector.wait_ge(win_sem, o + 1)
                    # fold the bank's 8 w' column groups into the window
                    # average (the 1/(sh*sw) scale is baked into pmat)
                    red = vector.tensor_reduce(
                        out=y_sb[:, j * C : (j + 1) * C],
                        in_=psum[:OUT, j * 512 : (j + 1) * 512].rearrange(
                            "p (w c) -> p c w", c=C
                        ),
                        axis=mybir.AxisListType.X,
                        op=mybir.AluOpType.add,
                    )
                    if o >= OUT - 2:
                        red.then_inc(mul_sem, 1)

    return nc


def _consts():
    hs, hsz = _windows(H, OUT)
    _, wsz = _windows(W, OUT)
    p = np.zeros((128, OUT * KH * OUT), np.float32)
    for j in range(OUT):
        for k in range(KH):
            n = j * KH + k
            for i in range(OUT):
                h0, h1 = int(hs[i]), int(hs[i] + hsz[i])
                for h in range(max(h0, k * 128), min(h1, (k + 1) * 128)):
                    p[h - k * 128, n * OUT + i] = 1.0 / (
                        float(hsz[i]) * float(wsz[j])
                    )
    return p.astype(ml_dtypes.bfloat16)


_NC_CACHE = None


def _run(x, **kwargs):
    global _NC_CACHE
    if _NC_CACHE is None:
        _NC_CACHE = _build()
    nc = _NC_CACHE
    p = _consts()
    x = np.ascontiguousarray(np.asarray(x, dtype=np.float32))
    in_maps = [
        {"x": x[b].reshape(H, W * C), "pmat": p}
        for b in range(N_CORES)
    ]
    res = run_bass_kernel_spmd(nc, in_maps, core_ids=list(range(N_CORES)), **kwargs)
    y = np.stack(
        [res.results[b]["out"].reshape(OUT, OUT, C) for b in range(N_CORES)]
    )
    return y, res


def kernel(x: np.ndarray) -> np.ndarray:
    y, _ = _run(x)
    return y
